# revision 2
# baseline (speedup 1.0000x reference)
"""Otsu binarization (nn_BinarizeLayer) on 8 Trainium2 NeuronCores.

Pipeline (data-parallel over batch, 2 images per core):
  L1 (device): RGB->gray (f32), per-partition min/max partials, gray kept in HBM
  host:        global min/max reduce (exact, f32)
  L2 (device): gray -> 256-bin index map (uint8) using floor((g-mn)/width)
  host:        bincount (uint16-pair trick) + Otsu between-class-variance argmax
  L3 (device): gray > thresh ? 1.0 : 0.0
"""

import time
import numpy as np
import concourse.bacc as bacc
import concourse.mybir as mybir
import concourse.tile as tile
from concourse.bass_utils import run_bass_kernel_spmd

N_CORES = 8
B, H, W, C = 16, 1024, 1024, 3
P = 128
FR = 1536              # raw f32 elems per partition-row per tile (512 pixels * 3ch)
FP = FR // 3           # gray pixels per row per tile
NT = (B * H * W // N_CORES) // (P * FP)   # 32 tiles per core
NBINS = 256

cR, cG, cB = np.float32(0.2989), np.float32(0.5870), np.float32(0.1140)

_cache = {}
stats = {}


def _build_l1():
    nc = bacc.Bacc(None, target_bir_lowering=False, debug=False)
    x = nc.dram_tensor("x", [NT, P, FR], mybir.dt.float32, kind="ExternalInput").ap()
    gray = nc.dram_tensor("gray", [NT, P, FP], mybir.dt.float32, kind="ExternalOutput").ap()
    pmin = nc.dram_tensor("pmin", [P, 1], mybir.dt.float32, kind="ExternalOutput").ap()
    pmax = nc.dram_tensor("pmax", [P, 1], mybir.dt.float32, kind="ExternalOutput").ap()

    kBG = float(cB / cG)
    kRG = float(cR / cG)
    with tile.TileContext(nc) as tc:
        with (
            tc.tile_pool(name="inp", bufs=3) as inp,
            tc.tile_pool(name="work", bufs=3) as work,
            tc.tile_pool(name="acc", bufs=1) as acc,
        ):
            mn_acc = acc.tile([P, NT], mybir.dt.float32)
            mx_acc = acc.tile([P, NT], mybir.dt.float32)
            for t in range(NT):
                tin = inp.tile([P, FR], mybir.dt.float32)
                nc.sync.dma_start(tin[:], x[t])
                v = tin[:].rearrange("p (n c) -> p n c", c=3)
                R, G, Bc = v[:, :, 0], v[:, :, 1], v[:, :, 2]

                t1 = work.tile([P, FP], mybir.dt.float32, tag="t1")
                nc.vector.scalar_tensor_tensor(
                    t1[:], Bc, kBG, G, mybir.AluOpType.mult, mybir.AluOpType.add
                )
                t2 = work.tile([P, FP], mybir.dt.float32, tag="t2")
                nc.vector.scalar_tensor_tensor(
                    t2[:], R, kRG, t1[:], mybir.AluOpType.mult, mybir.AluOpType.add
                )
                g = work.tile([P, FP], mybir.dt.float32, tag="g")
                nc.scalar.mul(g[:], t2[:], float(cG))

                nc.vector.tensor_reduce(
                    mn_acc[:, t : t + 1], g[:], mybir.AxisListType.X, mybir.AluOpType.min
                )
                nc.vector.tensor_reduce(
                    mx_acc[:, t : t + 1], g[:], mybir.AxisListType.X, mybir.AluOpType.max
                )
                nc.sync.dma_start(gray[t], g[:])

            mn1 = acc.tile([P, 1], mybir.dt.float32)
            mx1 = acc.tile([P, 1], mybir.dt.float32)
            nc.vector.tensor_reduce(
                mn1[:], mn_acc[:], mybir.AxisListType.X, mybir.AluOpType.min
            )
            nc.vector.tensor_reduce(
                mx1[:], mx_acc[:], mybir.AxisListType.X, mybir.AluOpType.max
            )
            nc.sync.dma_start(pmin[:], mn1[:])
            nc.sync.dma_start(pmax[:], mx1[:])
    nc.compile()
    return nc


def _build_l2():
    nc = bacc.Bacc(None, target_bir_lowering=False, debug=False)
    gray = nc.dram_tensor("gray", [NT, P, FP], mybir.dt.float32, kind="ExternalInput").ap()
    a_in = nc.dram_tensor("a", [P, 1], mybir.dt.float32, kind="ExternalInput").ap()
    s_in = nc.dram_tensor("s", [P, 1], mybir.dt.float32, kind="ExternalInput").ap()
    idx = nc.dram_tensor("idx", [NT, P, FP], mybir.dt.uint8, kind="ExternalOutput").ap()

    with tile.TileContext(nc) as tc:
        with (
            tc.tile_pool(name="inp", bufs=3) as inp,
            tc.tile_pool(name="work", bufs=3) as work,
            tc.tile_pool(name="consts", bufs=1) as consts,
        ):
            a_c = consts.tile([P, 1], mybir.dt.float32)
            s_c = consts.tile([P, 1], mybir.dt.float32)
            nc.sync.dma_start(a_c[:], a_in[:])
            nc.sync.dma_start(s_c[:], s_in[:])
            for t in range(NT):
                g = inp.tile([P, FP], mybir.dt.float32)
                nc.sync.dma_start(g[:], gray[t])
                # y = (g - a) * s   where a = mn + width/2 so that rint(y) = floor((g-mn)/width)
                y = work.tile([P, FP], mybir.dt.float32, tag="y")
                nc.vector.tensor_scalar(
                    out=y[:], in0=g[:], scalar1=a_c[:], scalar2=s_c[:],
                    op0=mybir.AluOpType.subtract, op1=mybir.AluOpType.mult,
                )
                iy = work.tile([P, FP], mybir.dt.int32, tag="iy")
                nc.vector.tensor_copy(iy[:], y[:])  # f32->i32 round-nearest-even
                u8 = work.tile([P, FP], mybir.dt.uint8, tag="u8")
                nc.vector.tensor_scalar(
                    out=u8[:], in0=iy[:], scalar1=0.0, scalar2=255.0,
                    op0=mybir.AluOpType.max, op1=mybir.AluOpType.min,
                )
                nc.sync.dma_start(idx[t], u8[:])
    nc.compile()
    return nc


def _build_l3():
    nc = bacc.Bacc(None, target_bir_lowering=False, debug=False)
    gray = nc.dram_tensor("gray", [NT, P, FP], mybir.dt.float32, kind="ExternalInput").ap()
    th_in = nc.dram_tensor("th", [P, 1], mybir.dt.float32, kind="ExternalInput").ap()
    out = nc.dram_tensor("out", [NT, P, FP], mybir.dt.float32, kind="ExternalOutput").ap()

    with tile.TileContext(nc) as tc:
        with (
            tc.tile_pool(name="inp", bufs=3) as inp,
            tc.tile_pool(name="work", bufs=3) as work,
            tc.tile_pool(name="consts", bufs=1) as consts,
        ):
            th_c = consts.tile([P, 1], mybir.dt.float32)
            nc.sync.dma_start(th_c[:], th_in[:])
            for t in range(NT):
                g = inp.tile([P, FP], mybir.dt.float32)
                nc.sync.dma_start(g[:], gray[t])
                o = work.tile([P, FP], mybir.dt.float32)
                nc.vector.tensor_scalar(
                    out=o[:], in0=g[:], scalar1=th_c[:], scalar2=0.0,
                    op0=mybir.AluOpType.is_gt, op1=mybir.AluOpType.bypass,
                )
                nc.sync.dma_start(out[t], o[:])
    nc.compile()
    return nc


def _get(name, builder):
    if name not in _cache:
        _cache[name] = builder()
    return _cache[name]


def _otsu_from_counts(counts_u, mn, mx):
    """Replicates the reference threshold computation (f32 semantics)."""
    f32 = np.float32
    counts = counts_u.astype(f32)
    width = f32((mx - mn) / f32(NBINS))
    centers = (mn + width * (np.arange(NBINS, dtype=f32) + f32(0.5))).astype(f32)
    w1 = np.cumsum(counts, dtype=f32)
    w2 = np.cumsum(counts[::-1], dtype=f32)[::-1]
    cc = (counts * centers).astype(f32)
    s1 = np.cumsum(cc, dtype=f32)
    s2 = np.cumsum(cc[::-1], dtype=f32)[::-1]
    m1 = (s1 / np.maximum(w1, f32(1.0))).astype(f32)
    m2 = (s2 / np.maximum(w2, f32(1.0))).astype(f32)
    var12 = (w1[:-1] * w2[1:] * (m1[:-1] - m2[1:]) ** 2).astype(f32)
    k = int(np.argmax(var12))
    return centers[k], k, var12


def kernel(inputs):
    x = np.ascontiguousarray(np.asarray(inputs), dtype=np.float32)
    assert x.shape == (B, H, W, C)
    core_ids = list(range(N_CORES))
    shards = x.reshape(N_CORES, NT, P, FR)

    l1 = _get("l1", _build_l1)
    l2 = _get("l2", _build_l2)
    l3 = _get("l3", _build_l3)

    t0 = time.perf_counter()
    r1 = run_bass_kernel_spmd(l1, [{"x": shards[c]} for c in core_ids], core_ids)
    t1 = time.perf_counter()
    grays = [r1.results[c]["gray"] for c in core_ids]
    mn = np.float32(min(r1.results[c]["pmin"].min() for c in core_ids))
    mx = np.float32(max(r1.results[c]["pmax"].max() for c in core_ids))

    f32 = np.float32
    width = f32((mx - mn) / f32(NBINS))
    a = f32(mn + width * f32(0.5))       # shift by half-bin: rint(y) == floor
    s = f32(f32(1.0) / width)
    a_col = np.full((P, 1), a, np.float32)
    s_col = np.full((P, 1), s, np.float32)

    t2 = time.perf_counter()
    r2 = run_bass_kernel_spmd(
        l2,
        [{"gray": grays[c], "a": a_col, "s": s_col} for c in core_ids],
        core_ids,
    )
    t3 = time.perf_counter()

    idx_all = np.concatenate([r2.results[c]["idx"].reshape(-1) for c in core_ids])
    # fast uint8 bincount via uint16-pair trick
    c16 = np.bincount(idx_all.view(np.uint16), minlength=65536).reshape(256, 256)
    counts = c16.sum(0) + c16.sum(1)

    thresh, k, var12 = _otsu_from_counts(counts, mn, mx)
    th_col = np.full((P, 1), thresh, np.float32)

    t4 = time.perf_counter()
    r3 = run_bass_kernel_spmd(
        l3, [{"gray": grays[c], "th": th_col} for c in core_ids], core_ids
    )
    t5 = time.perf_counter()

    out = np.concatenate([r3.results[c]["out"].reshape(-1) for c in core_ids])
    stats.update(
        l1_s=t1 - t0, l2_s=t3 - t2, l3_s=t5 - t4,
        host_s=(t2 - t1) + (t4 - t3),
        mn=float(mn), mx=float(mx), thresh=float(thresh), k=k,
        counts=counts, var12=var12,
    )
    return out.reshape(B, H, W, 1)


# revision 4
# speedup vs baseline: 2.2238x; 2.2238x over previous
"""Otsu binarization (nn_BinarizeLayer) on 8 Trainium2 NeuronCores.

Single fused device launch (data-parallel over batch, 2 images per core):
  phase A: RGB->gray accumulation in SBUF (t2 = gray/cG stays resident,
           never touches HBM), per-partition min/max partials
  on-device: partition_all_reduce + 8-core AllReduce(max) of (-min, max),
           then the f32 scalar chain producing the fine-bin affine
  phase B: j = clip(rint((t2 - a)*s), 0, 511)  -- 512 fine bins (2 per
           histogram bin) -> uint16 map to HBM.  j>>1 is the Otsu bin;
           the Otsu threshold is exactly the boundary between fine bins
           2k* and 2k*+1, so the final output is just (j > 2k*).
host:      bincount(j) -> 256-bin histogram -> Otsu argmax (f32,
           replicating the reference semantics) -> out = (j > 2k*).

Device traffic per core: 24 MiB in + 4 MiB out  (~memory roofline).
"""

import time
import numpy as np
import concourse.bacc as bacc
import concourse.mybir as mybir
import concourse.tile as tile
from concourse import bass_isa
from concourse.bass_utils import run_bass_kernel_spmd

N_CORES = 8
B, H, W, C = 16, 1024, 1024, 3
P = 128
FR = 1536              # raw f32 elems per partition-row per tile (512 px * 3ch)
FP = FR // 3           # gray pixels per row per tile
NT = (B * H * W // N_CORES) // (P * FP)   # 32 tiles per core
NBINS = 256
CLIP_HI = 511.49       # rint() of this is 511 = max fine bin

cR, cG, cB = np.float32(0.2989), np.float32(0.5870), np.float32(0.1140)

_cache = {}
stats = {}

AL = mybir.AluOpType
AX = mybir.AxisListType
F32 = mybir.dt.float32


def _build_v2():
    nc = bacc.Bacc(None, target_bir_lowering=False, debug=False)
    x = nc.dram_tensor("x", [NT, P, FR], F32, kind="ExternalInput").ap()
    jout = nc.dram_tensor("j", [NT, P, FP], mybir.dt.uint16, kind="ExternalOutput").ap()
    mnmx = nc.dram_tensor("mnmx", [1, 2], F32, kind="ExternalOutput").ap()

    kBG = float(cB / cG)
    kRG = float(cR / cG)
    with tile.TileContext(nc) as tc:
        with (
            tc.tile_pool(name="inp", bufs=3) as inp,
            tc.tile_pool(name="work", bufs=3) as work,
            tc.tile_pool(name="res", bufs=1) as res,
            tc.tile_pool(name="sca", bufs=1) as sca,
            tc.tile_pool(name="dram", bufs=1, space="DRAM") as dram,
        ):
            T2 = res.tile([P, NT * FP], F32)       # resident gray/cG
            nacc = res.tile([P, NT], F32)          # per-tile min cols
            xacc = res.tile([P, NT], F32)          # per-tile max cols

            # ---------------- phase A ----------------
            for t in range(NT):
                tin = inp.tile([P, FR], F32)
                nc.sync.dma_start(tin[:], x[t])
                v = tin[:].rearrange("p (n c) -> p n c", c=3)
                R, G, Bc = v[:, :, 0], v[:, :, 1], v[:, :, 2]

                Rs = work.tile([P, FP], F32, tag="Rs")
                nc.scalar.activation(Rs[:], R, mybir.ActivationFunctionType.Copy,
                                     bias=0.0, scale=kRG)
                t1 = work.tile([P, FP], F32, tag="t1")
                nc.vector.scalar_tensor_tensor(t1[:], Bc, kBG, G, AL.mult, AL.add)
                t2s = T2[:, t * FP : (t + 1) * FP]
                nc.gpsimd.tensor_tensor(t2s, Rs[:], t1[:], AL.add)

                nc.vector.tensor_reduce(nacc[:, t : t + 1], t2s, AX.X, AL.min)
                nc.vector.tensor_reduce(xacc[:, t : t + 1], t2s, AX.X, AL.max)

            # ---------------- min/max -> AllReduce ----------------
            nmm = sca.tile([P, 2], F32)
            mn1 = sca.tile([P, 1], F32)
            nc.vector.tensor_reduce(mn1[:], nacc[:], AX.X, AL.min)
            nc.vector.tensor_single_scalar(nmm[:, 0:1], mn1[:], -1.0, AL.mult)
            nc.vector.tensor_reduce(nmm[:, 1:2], xacc[:], AX.X, AL.max)
            pmm = sca.tile([P, 2], F32)
            nc.gpsimd.partition_all_reduce(pmm[:], nmm[:], 128, bass_isa.ReduceOp.max)

            arin = dram.tile([1, 2], F32)
            arout = dram.tile([1, 2], F32)
            nc.sync.dma_start(arin[:], pmm[0:1, :])
            nc.gpsimd.collective_compute(
                "AllReduce", AL.max,
                replica_groups=[list(range(N_CORES))],
                ins=[arin.opt()], outs=[arout.opt()],
            )
            ar = sca.tile([1, 2], F32)             # [-min_t2, max_t2] global
            nc.sync.dma_start(ar[:], arout[:])

            # ---------------- scalar chain (t2-domain affine) ----------------
            gmm = sca.tile([1, 2], F32)            # [mn_g, mx_g]
            nc.vector.tensor_single_scalar(gmm[:, 0:1], ar[:, 0:1], -float(cG), AL.mult)
            nc.vector.tensor_single_scalar(gmm[:, 1:2], ar[:, 1:2], float(cG), AL.mult)
            nc.sync.dma_start(mnmx[:], gmm[:])

            t_ = sca.tile([1, 8], F32)             # scratch scalars
            wid, q, a3, a3t, hw2, s3, s3t, nb = (t_[:, i : i + 1] for i in range(8))
            d = sca.tile([1, 1], F32)
            nc.vector.tensor_tensor(d[:], gmm[:, 1:2], gmm[:, 0:1], AL.subtract)
            nc.vector.tensor_single_scalar(wid, d[:], 1.0 / 256.0, AL.mult)   # exact
            nc.vector.tensor_single_scalar(q, wid, 0.25, AL.mult)             # exact
            nc.vector.tensor_tensor(a3, gmm[:, 0:1], q, AL.add)               # mn + w/4
            nc.vector.tensor_single_scalar(a3t, a3, float(1.0 / cG), AL.mult)
            nc.vector.tensor_single_scalar(hw2, wid, 0.5, AL.mult)            # exact
            nc.vector.reciprocal(s3, hw2)                                     # 2/width
            nc.vector.tensor_single_scalar(s3t, s3, float(cG), AL.mult)
            nc.vector.tensor_tensor(nb, a3t, s3t, AL.mult)
            sb2 = sca.tile([1, 2], F32)
            nc.vector.tensor_copy(sb2[:, 0:1], s3t)
            nc.vector.tensor_single_scalar(sb2[:, 1:2], nb, -1.0, AL.mult)
            bc2 = sca.tile([P, 2], F32)
            nc.gpsimd.partition_broadcast(bc2[:], sb2[:])
            scol, bcol = bc2[:, 0:1], bc2[:, 1:2]

            # ---------------- phase B ----------------
            for t in range(NT):
                t2s = T2[:, t * FP : (t + 1) * FP]
                ju = work.tile([P, FP], mybir.dt.uint16, tag="ju")
                nc.scalar.activation(
                    ju[:], t2s, mybir.ActivationFunctionType.Relu,
                    bias=bcol, scale=scol,
                )
                nc.sync.dma_start(jout[t], ju[:])
    nc.compile()
    return nc


def _get(name, builder):
    if name not in _cache:
        _cache[name] = builder()
    return _cache[name]


def _otsu_from_counts(counts_u, mn, mx):
    """Replicates the reference threshold computation (f32 semantics)."""
    f32 = np.float32
    counts = counts_u.astype(f32)
    width = f32((mx - mn) / f32(NBINS))
    centers = (mn + width * (np.arange(NBINS, dtype=f32) + f32(0.5))).astype(f32)
    w1 = np.cumsum(counts, dtype=f32)
    w2 = np.cumsum(counts[::-1], dtype=f32)[::-1]
    cc = (counts * centers).astype(f32)
    s1 = np.cumsum(cc, dtype=f32)
    s2 = np.cumsum(cc[::-1], dtype=f32)[::-1]
    m1 = (s1 / np.maximum(w1, f32(1.0))).astype(f32)
    m2 = (s2 / np.maximum(w2, f32(1.0))).astype(f32)
    var12 = (w1[:-1] * w2[1:] * (m1[:-1] - m2[1:]) ** 2).astype(f32)
    k = int(np.argmax(var12))
    return centers[k], k, var12


def kernel(inputs):
    x = np.ascontiguousarray(np.asarray(inputs), dtype=np.float32)
    assert x.shape == (B, H, W, C)
    core_ids = list(range(N_CORES))
    shards = x.reshape(N_CORES, NT, P, FR)

    v2 = _get("v2", _build_v2)

    t0 = time.perf_counter()
    r = run_bass_kernel_spmd(v2, [{"x": shards[c]} for c in core_ids], core_ids)
    t1 = time.perf_counter()

    mn, mx = (np.float32(v) for v in r.results[0]["mnmx"][0])
    j = np.stack([r.results[c]["j"] for c in core_ids])  # (8, NT, P, FP)

    cj = np.bincount(j.ravel(), minlength=65536)
    counts = cj[0:512:2] + cj[1:512:2]
    counts[255] += cj[512:].sum()   # rint overflow of the top fine bin

    thresh, k, var12 = _otsu_from_counts(counts, mn, mx)

    out = (j.reshape(-1) > np.uint16(2 * k)).astype(np.float32)
    t2 = time.perf_counter()

    stats.update(
        launch_s=t1 - t0, host_s=t2 - t1,
        mn=float(mn), mx=float(mx), thresh=float(thresh), k=k,
        counts=counts, var12=var12,
    )
    return out.reshape(B, H, W, 1)


# revision 6
# speedup vs baseline: 2.2561x; 1.0145x over previous
"""Otsu binarization (nn_BinarizeLayer) on 8 Trainium2 NeuronCores.

Single fused device launch (data-parallel over batch, 2 images per core):
  phase A: RGB->gray accumulation in SBUF (t2 = gray/cG stays resident,
           never touches HBM), per-partition min/max partials
  on-device: partition_all_reduce + 8-core AllReduce(max) of (-min, max),
           then the f32 scalar chain producing the fine-bin affine
  phase B: j = clip(rint((t2 - a)*s), 0, 511)  -- 512 fine bins (2 per
           histogram bin) -> uint16 map to HBM.  j>>1 is the Otsu bin;
           the Otsu threshold is exactly the boundary between fine bins
           2k* and 2k*+1, so the final output is just (j > 2k*).
host:      bincount(j) -> 256-bin histogram -> Otsu argmax (f32,
           replicating the reference semantics) -> out = (j > 2k*).

Device traffic per core: 24 MiB in + 4 MiB out  (~memory roofline).
"""

import time
import numpy as np
import concourse.bacc as bacc
import concourse.mybir as mybir
import concourse.tile as tile
from concourse import bass_isa
from concourse.bass_utils import run_bass_kernel_spmd

N_CORES = 8
B, H, W, C = 16, 1024, 1024, 3
P = 128
FR = 1536              # raw f32 elems per partition-row per tile (512 px * 3ch)
FP = FR // 3           # gray pixels per row per tile
NT = (B * H * W // N_CORES) // (P * FP)   # 32 tiles per core
NBINS = 256
RED_CHUNK = 2          # tiles per min/max reduce instruction

cR, cG, cB = np.float32(0.2989), np.float32(0.5870), np.float32(0.1140)

_cache = {}
stats = {}

AL = mybir.AluOpType
AX = mybir.AxisListType
F32 = mybir.dt.float32


def _build_v2():
    nc = bacc.Bacc(None, target_bir_lowering=False, debug=False)
    x = nc.dram_tensor("x", [NT, P, FR], F32, kind="ExternalInput").ap()
    jout = nc.dram_tensor("j", [NT, P, FP], mybir.dt.uint16, kind="ExternalOutput").ap()
    mnmx = nc.dram_tensor("mnmx", [1, 2], F32, kind="ExternalOutput").ap()

    kBG = float(cB / cG)
    kRG = float(cR / cG)
    with tile.TileContext(nc) as tc:
        with (
            tc.tile_pool(name="inp", bufs=3) as inp,
            tc.tile_pool(name="work", bufs=4) as work,
            tc.tile_pool(name="res", bufs=1) as res,
            tc.tile_pool(name="sca", bufs=1) as sca,
            tc.tile_pool(name="dram", bufs=1, space="DRAM") as dram,
        ):
            T2 = res.tile([P, NT * FP], F32)       # resident gray/cG
            NCH = NT // RED_CHUNK
            nacc = res.tile([P, NCH], F32)         # per-chunk min cols
            xacc = res.tile([P, NCH], F32)         # per-chunk max cols

            # ---------------- phase A ----------------
            # ACT scales B, GPSIMD adds G, DVE does the final scaled add of R
            # (so the chunked min/max reduces depend only on DVE's own output
            # -- no cross-engine bubble in the in-order DVE queue).
            for t in range(NT):
                tin = inp.tile([P, FR], F32)
                nc.sync.dma_start(tin[:], x[t])
                v = tin[:].rearrange("p (n c) -> p n c", c=3)
                R, G, Bc = v[:, :, 0], v[:, :, 1], v[:, :, 2]

                Bs = work.tile([P, FP], F32, tag="Bs")
                nc.scalar.activation(Bs[:], Bc, mybir.ActivationFunctionType.Copy,
                                     bias=0.0, scale=kBG)
                t1 = work.tile([P, FP], F32, tag="t1")
                nc.gpsimd.tensor_tensor(t1[:], Bs[:], G, AL.add)
                t2s = T2[:, t * FP : (t + 1) * FP]
                nc.vector.scalar_tensor_tensor(t2s, R, kRG, t1[:], AL.mult, AL.add)

                if (t + 1) % RED_CHUNK == 0:
                    c = t // RED_CHUNK
                    span = T2[:, c * RED_CHUNK * FP : (c + 1) * RED_CHUNK * FP]
                    nc.vector.tensor_reduce(nacc[:, c : c + 1], span, AX.X, AL.min)
                    nc.vector.tensor_reduce(xacc[:, c : c + 1], span, AX.X, AL.max)

            # ------- min/max -> AllReduce (partition partials ride inside) -------
            nmm = sca.tile([P, 2], F32)            # [-min, max] per partition
            mn1 = sca.tile([P, 1], F32)
            nc.vector.tensor_reduce(mn1[:], nacc[:], AX.X, AL.min)
            nc.vector.tensor_single_scalar(nmm[:, 0:1], mn1[:], -1.0, AL.mult)
            nc.vector.tensor_reduce(nmm[:, 1:2], xacc[:], AX.X, AL.max)

            arin = dram.tile([1, 2 * P], F32)
            arout = dram.tile([1, 2 * P], F32)
            nc.sync.dma_start(arin[:], nmm[:])     # flatten partitions into free
            nc.gpsimd.collective_compute(
                "AllReduce", AL.max,
                replica_groups=[list(range(N_CORES))],
                ins=[arin.opt()], outs=[arout.opt()],
            )
            # replicate the AR result to every partition (0-stride DMA read)
            arbc = sca.tile([P, 2 * P], F32)
            nc.sync.dma_start(arbc[:], arout[:][0].partition_broadcast(P))

            # ---------------- scalar chain, redundantly on all partitions ----------------
            v2 = arbc[:].rearrange("p (n c) -> p n c", c=2)
            t_ = sca.tile([P, 12], F32)            # scratch scalar columns
            (nmg, mxg, mn_g, mx_g, d, wid, q, a3, a3t, hw2, s3t, nb) = (
                t_[:, i : i + 1] for i in range(12)
            )
            s3 = sca.tile([P, 1], F32)
            nc.vector.tensor_reduce(nmg, v2[:, :, 0], AX.X, AL.max)   # -min_t2
            nc.vector.tensor_reduce(mxg, v2[:, :, 1], AX.X, AL.max)   # max_t2
            nc.vector.tensor_single_scalar(mn_g, nmg, -float(cG), AL.mult)
            nc.vector.tensor_single_scalar(mx_g, mxg, float(cG), AL.mult)
            nc.sync.dma_start(mnmx[:], t_[0:1, 2:4])
            nc.vector.tensor_tensor(d, mx_g, mn_g, AL.subtract)
            nc.vector.tensor_single_scalar(wid, d, 1.0 / 256.0, AL.mult)   # exact
            nc.vector.tensor_single_scalar(q, wid, 0.25, AL.mult)          # exact
            nc.vector.tensor_tensor(a3, mn_g, q, AL.add)                   # mn + w/4
            nc.vector.tensor_single_scalar(a3t, a3, float(1.0 / cG), AL.mult)
            nc.vector.tensor_single_scalar(hw2, wid, 0.5, AL.mult)         # exact
            nc.vector.reciprocal(s3[:], hw2)                               # 2/width
            nc.vector.tensor_single_scalar(s3t, s3[:], float(cG), AL.mult)
            nc.vector.tensor_tensor(nb, a3t, s3t, AL.mult)
            bcol_t = sca.tile([P, 1], F32)
            nc.vector.tensor_single_scalar(bcol_t[:], nb, -1.0, AL.mult)
            scol, bcol, a3tcol = s3t, bcol_t[:], a3t

            # ---------------- phase B (split ACT / DVE) ----------------
            for t in range(NT):
                t2s = T2[:, t * FP : (t + 1) * FP]
                if t % 2 == 0:
                    ju = work.tile([P, FP], mybir.dt.uint16, tag="jA")
                    nc.scalar.activation(
                        ju[:], t2s, mybir.ActivationFunctionType.Relu,
                        bias=bcol, scale=scol,
                    )
                else:
                    yv = work.tile([P, FP], F32, tag="yv")
                    nc.vector.tensor_scalar(
                        out=yv[:], in0=t2s, scalar1=a3tcol, scalar2=scol,
                        op0=AL.subtract, op1=AL.mult,
                    )
                    ju = work.tile([P, FP], mybir.dt.uint16, tag="jD")
                    nc.vector.tensor_single_scalar(ju[:], yv[:], 0.0, AL.max)
                nc.sync.dma_start(jout[t], ju[:])
    nc.compile()
    return nc


def _get(name, builder):
    if name not in _cache:
        _cache[name] = builder()
    return _cache[name]


def _otsu_from_counts(counts_u, mn, mx):
    """Replicates the reference threshold computation (f32 semantics)."""
    f32 = np.float32
    counts = counts_u.astype(f32)
    width = f32((mx - mn) / f32(NBINS))
    centers = (mn + width * (np.arange(NBINS, dtype=f32) + f32(0.5))).astype(f32)
    w1 = np.cumsum(counts, dtype=f32)
    w2 = np.cumsum(counts[::-1], dtype=f32)[::-1]
    cc = (counts * centers).astype(f32)
    s1 = np.cumsum(cc, dtype=f32)
    s2 = np.cumsum(cc[::-1], dtype=f32)[::-1]
    m1 = (s1 / np.maximum(w1, f32(1.0))).astype(f32)
    m2 = (s2 / np.maximum(w2, f32(1.0))).astype(f32)
    var12 = (w1[:-1] * w2[1:] * (m1[:-1] - m2[1:]) ** 2).astype(f32)
    k = int(np.argmax(var12))
    return centers[k], k, var12


def kernel(inputs):
    x = np.ascontiguousarray(np.asarray(inputs), dtype=np.float32)
    assert x.shape == (B, H, W, C)
    core_ids = list(range(N_CORES))
    shards = x.reshape(N_CORES, NT, P, FR)

    v2 = _get("v2", _build_v2)

    t0 = time.perf_counter()
    r = run_bass_kernel_spmd(v2, [{"x": shards[c]} for c in core_ids], core_ids)
    t1 = time.perf_counter()

    mn, mx = (np.float32(v) for v in r.results[0]["mnmx"][0])
    j = np.stack([r.results[c]["j"] for c in core_ids])  # (8, NT, P, FP)

    cj = np.bincount(j.ravel(), minlength=65536)
    counts = cj[0:512:2] + cj[1:512:2]
    counts[255] += cj[512:].sum()   # rint overflow of the top fine bin

    thresh, k, var12 = _otsu_from_counts(counts, mn, mx)

    out = (j.reshape(-1) > np.uint16(2 * k)).astype(np.float32)
    t2 = time.perf_counter()

    stats.update(
        launch_s=t1 - t0, host_s=t2 - t1,
        mn=float(mn), mx=float(mx), thresh=float(thresh), k=k,
        counts=counts, var12=var12,
    )
    return out.reshape(B, H, W, 1)
